# revision 19
# baseline (speedup 1.0000x reference)
"""CustomLSTM cell (4 gated projections + cell update) on 8 TRN2 NeuronCores.

Data-parallel over the batch dim: each core processes B/8 = 4096 rows.
Per core, z = x @ [Wi|Wf|Wg|Wo] is computed as bf16 matmuls accumulating
f32 into 4 PSUM banks (N = 4*512), the gate nonlinearities run on the
scalar engine straight out of PSUM, and the cell/hidden updates run on
the vector engine.  Host-side prep lays x out transposed ([p, ko, b]
per 512-row batch group) so every DMA is contiguous per partition, and
casts x/W to bf16 (PSUM accumulation stays f32).

Self-contained: shapes/sharding hardcoded for
input [32768, 1024], cell_state [32768, 512], W* [1024, 512].
"""

import os

import numpy as np
import ml_dtypes

import bass_rust
import concourse.bass as bass
import concourse.mybir as mybir
import concourse.tile as tile
from concourse.bass_utils import run_bass_kernel_spmd

N_CORES = 8
B = 32768
D = 1024
H = 512
P = 128
B_LOC = B // N_CORES        # 4096 rows per core
KO = D // P                 # 8 k-subtiles
NW = 4 * H                  # 2048 concatenated gate dim
NG = NW // H                # 4 psum banks of 512
BG_ROWS = 512               # batch rows per x slab
BG = B_LOC // BG_ROWS       # 8 slabs per core
BT_PER_BG = BG_ROWS // P    # 4 batch tiles per slab

BF16 = mybir.dt.bfloat16
F32 = mybir.dt.float32

# Filled by the last kernel() call: BassKernelResults (exec_time_ns etc).
LAST_RESULTS = None
_CACHED = {}


def _split_multi_waits(nc):
    """Legalize for a walrus build that accepts one sync-wait per instruction.

    Tile's wait assignment attaches every needed sem wait to the consuming
    instruction; this backend rejects >1 ("Too many sync wait commands").
    Move all but the last wait onto dedicated NoOps inserted just before the
    instruction on the same engine queue — sequential waits on one engine are
    equivalent to a single multi-wait instruction for monotone sem waits.
    """
    n = 0
    for f in nc.m.functions:
        for blk in f.blocks:
            insts = blk.instructions
            if not any(
                i.sync_info is not None and len(i.sync_info.on_wait) > 1
                for i in insts
            ):
                continue
            out = []
            for inst in insts:
                si = inst.sync_info
                if si is not None and len(si.on_wait) > 1:
                    waits = list(si.on_wait)
                    for w in waits[:-1]:
                        nop = mybir.InstNoOp(name=f"waitsplit_{n}", ins=[], outs=[])
                        n += 1
                        nop.engine = inst.engine
                        nop.sync_info = bass_rust.SyncInfo(on_wait=[w], on_update=[])
                        out.append(nop)
                    inst.sync_info = bass_rust.SyncInfo(
                        on_wait=[waits[-1]], on_update=list(si.on_update)
                    )
                out.append(inst)
            blk.instructions = out


class _FastTailTileContext(tile.TileContext):
    """Drop the second tail all-engine barrier.

    The stock tail is [drain+waits][barrier][sem/queue reset][barrier]; the
    final barrier only isolates the gpsimd-side reset from code that would
    follow it — nothing follows it here, and NRT waits for every engine
    stream (including gpsimd's reset) to halt before completion, so engines
    can end right after the first barrier. Saves ~4-6us of EVSEM ring.
    """

    def _drain_and_barrier(self, tick_clock, wait_clock):
        from concourse.vector_clock import ScopedClock

        drain_inst = self.nc.sync.drain()
        wait_clock.add_sem_waits(
            drain_inst.ins, ScopedClock({None: tick_clock.global_clock})
        )
        self.nc.all_engine_barrier()
        assert self.sems is not None
        popped = self.nc._tile_sem_poison_stack.pop()
        assert popped is self._sem_poison
        self.nc.clear_and_free_semaphores(list(self.sems.allocated().values()))


def _build(with_bias):
    nc = bass.Bass()
    AF = mybir.ActivationFunctionType
    ts = bass.ts

    xt = nc.dram_tensor("xt", [BG, P, KO, BG_ROWS], BF16, kind="ExternalInput")
    w = nc.dram_tensor("w", [P, KO, NW], BF16, kind="ExternalInput")
    cell = nc.dram_tensor("cell", [B_LOC, H], F32, kind="ExternalInput")
    if with_bias:
        bias = nc.dram_tensor("bias", [P, NW], F32, kind="ExternalInput")
    h_out = nc.dram_tensor("h_out", [B_LOC, H], F32, kind="ExternalOutput")
    c_out = nc.dram_tensor("c_out", [B_LOC, H], F32, kind="ExternalOutput")

    with _FastTailTileContext(nc) as tc:
        with (
            tc.tile_pool(name="wpool", bufs=1) as wpool,
            tc.tile_pool(name="xpool", bufs=2) as xpool,
            tc.tile_pool(name="cpool", bufs=4) as cpool,
            tc.tile_pool(name="gpool", bufs=3) as gpool,
            tc.tile_pool(name="ppool", bufs=8, space="PSUM") as ppool,
        ):
            bias_t = None
            if with_bias:
                bias_t = wpool.tile([P, NW], F32, tag="bias_t", name="bias_t")
                nc.sync.dma_start(bias_t[:], bias[:])

            def epilogue(ps, ct, rows, uid, splits=1):
                # gates from psum -> cell/hidden update -> DMA out.
                # splits>1 pipelines the serial ACT->DVE->DMA chain in column
                # chunks — used for the last batch tile to shorten the tail.
                if with_bias:
                    zs = []
                    for nn in range(NG):
                        z = gpool.tile([P, H], F32, tag=f"z{nn}", name=f"z{nn}_{uid}")
                        nc.vector.tensor_add(z[:], ps[nn], bias_t[:, ts(nn, H)])
                        zs.append(z)
                else:
                    zs = ps
                w_ = H // splits
                for q in range(splits):
                    cs = slice(q * w_, (q + 1) * w_)
                    i_t = gpool.tile([P, w_], F32, tag="i_t", name=f"i_{uid}_{q}")
                    nc.scalar.activation(i_t[:], zs[0][:, cs], AF.Sigmoid)
                    f_t = gpool.tile([P, w_], F32, tag="f_t", name=f"f_{uid}_{q}")
                    nc.scalar.activation(f_t[:], zs[1][:, cs], AF.Sigmoid)
                    g_t = gpool.tile([P, w_], F32, tag="g_t", name=f"g_{uid}_{q}")
                    nc.scalar.activation(g_t[:], zs[2][:, cs], AF.Tanh)
                    o_t = gpool.tile([P, w_], F32, tag="o_t", name=f"o_{uid}_{q}")
                    nc.scalar.activation(o_t[:], zs[3][:, cs], AF.Sigmoid)

                    fc = gpool.tile([P, w_], F32, tag="fc", name=f"fc_{uid}_{q}")
                    nc.vector.tensor_mul(fc[:], f_t[:], ct[:, cs])
                    ig = gpool.tile([P, w_], F32, tag="ig", name=f"ig_{uid}_{q}")
                    nc.vector.tensor_mul(ig[:], i_t[:], g_t[:])
                    cn = gpool.tile([P, w_], F32, tag="cn", name=f"cn_{uid}_{q}")
                    nc.vector.tensor_add(cn[:], fc[:], ig[:])
                    tn = gpool.tile([P, w_], F32, tag="tn", name=f"tn_{uid}_{q}")
                    nc.scalar.activation(tn[:], cn[:], AF.Tanh)
                    hn = gpool.tile([P, w_], F32, tag="hn", name=f"hn_{uid}_{q}")
                    nc.vector.tensor_mul(hn[:], o_t[:], tn[:])

                    nc.sync.dma_start(c_out[rows, cs], cn[:])
                    nc.sync.dma_start(h_out[rows, cs], hn[:])

            # PE warmup: ~3.4us of zero matmuls with no DMA dependency so the
            # HAM clock gate opens to 2.4GHz while the first loads are in
            # flight. Results land in a psum slot that bt0 overwrites.
            wz = wpool.tile([P, P], BF16, tag="wz", name="wz")
            nc.gpsimd.memset(wz[:], 0.0)
            warm_ps = ppool.tile([P, P], F32, tag="ps", name="warm_ps")
            for _ in range(16):
                nc.tensor.matmul(warm_ps[:], wz[:], wz[:], start=True, stop=True)

            # Startup DMAs in consumption order: the single HW DMA stream
            # delivers ~356GB/s in trigger order (and each trigger costs
            # ~0.6us on the Sync queue, so fewer/bigger is better past the
            # first chunk). xs0_k rides ahead of its W chunk so the first
            # LDWEIGHTS fires as early as possible.
            # Startup DMAs in consumption order. The HWDGE queues share HBM
            # bandwidth round-robin, so the first W chunk lands ~3us after
            # its trigger; the x chunks ride the (otherwise idle) ACT
            # trigger queue so Sync's serial trigger stream only carries W.
            # k=0 W is split per-bank so the very first matmul only needs
            # 256KB (xs0_0 + one piece) in flight — it lands right as the
            # warmup stream drains, keeping the PE clock-gate open.
            wks, xs0, w0p = [], [], []
            for k in range(KO):
                x0k = wpool.tile([P, BG_ROWS], BF16, tag=f"xs0_{k}", name=f"xs0_{k}")
                nc.scalar.dma_start(x0k[:], xt[0, :, k, :])
                xs0.append(x0k)
                if k == 0:
                    for nn in range(NG):
                        wp = wpool.tile([P, H], BF16, tag=f"w0p{nn}", name=f"w0p{nn}")
                        nc.sync.dma_start(wp[:], w[:, 0, ts(nn, H)])
                        w0p.append(wp)
                    wks.append(None)
                else:
                    wk = wpool.tile([P, NW], BF16, tag=f"w{k}", name=f"w{k}")
                    nc.sync.dma_start(wk[:], w[:, k, :])
                    wks.append(wk)

            def w_slice(k, nn):
                if k == 0:
                    return w0p[nn][:]
                return wks[k][:, ts(nn, H)]

            # Slab 0: k-major over j-pairs so PE consumes each W chunk as it
            # arrives instead of stalling for the whole 4MB of W.
            cts0 = []
            for j in range(BT_PER_BG):
                ct = cpool.tile([P, H], F32, tag="ct", name=f"ct0_{j}")
                nc.sync.dma_start(ct[:], cell[j * P : (j + 1) * P, :])
                cts0.append(ct)
            for jp in (0, 2):
                ps2 = {
                    (j, nn): ppool.tile([P, H], F32, tag="ps", name=f"ps0_{j}_{nn}")
                    for j in (jp, jp + 1)
                    for nn in range(NG)
                }
                for k in range(KO):
                    for j in (jp, jp + 1):
                        lhsT = xs0[k][:, ts(j, P)]
                        for nn in range(NG):
                            nc.tensor.matmul(
                                ps2[(j, nn)],
                                lhsT,
                                w_slice(k, nn),
                                start=(k == 0),
                                stop=(k == KO - 1),
                            )
                for j in (jp, jp + 1):
                    epilogue(
                        [ps2[(j, nn)] for nn in range(NG)],
                        cts0[j],
                        slice(j * P, (j + 1) * P),
                        f"g0_{j}",
                    )

            # Slabs 1..7: j-major, full-rate PE against prefetched slabs.
            for g in range(1, BG):
                xs = xpool.tile([P, KO, BG_ROWS], BF16, tag="xs", name="xs")
                nc.sync.dma_start(xs[:], xt[g])
                for j in range(BT_PER_BG):
                    bt = g * BT_PER_BG + j
                    rows = slice(bt * P, (bt + 1) * P)
                    ct = cpool.tile([P, H], F32, tag="ct", name=f"ct_{bt}")
                    nc.sync.dma_start(ct[:], cell[rows, :])
                    ps = [
                        ppool.tile([P, H], F32, tag="ps", name=f"ps{nn}_{bt}")
                        for nn in range(NG)
                    ]
                    last_bt = g == BG - 1 and j == BT_PER_BG - 1
                    if last_bt:
                        # Bank-by-bank (k-inner) so gates g/i/f are ready
                        # before the final o-bank matmul: the post-MM tail
                        # shrinks to sigmoid(o) -> h -> DMA.
                        for nn in (2, 0, 1, 3):
                            for k in range(KO):
                                nc.tensor.matmul(
                                    ps[nn],
                                    xs[:, k, ts(j, P)],
                                    w_slice(k, nn),
                                    start=(k == 0),
                                    stop=(k == KO - 1),
                                )
                    else:
                        for k in range(KO):
                            lhsT = xs[:, k, ts(j, P)]
                            for nn in range(NG):
                                nc.tensor.matmul(
                                    ps[nn],
                                    lhsT,
                                    w_slice(k, nn),
                                    start=(k == 0),
                                    stop=(k == KO - 1),
                                )
                    epilogue(ps, ct, rows, f"g{g}_{j}")

    _split_multi_waits(nc)
    return nc


def kernel(input, cell_state, Wi, bi, Wf, bf, Wg, bg, Wo, bo):
    global LAST_RESULTS

    x = np.asarray(input, dtype=np.float32)
    cell = np.ascontiguousarray(np.asarray(cell_state, dtype=np.float32))
    Wcat = np.concatenate(
        [np.asarray(m, dtype=np.float32) for m in (Wi, Wf, Wg, Wo)], axis=1
    )  # [D, 4H]
    bcat = np.concatenate(
        [np.asarray(v, dtype=np.float32) for v in (bi, bf, bg, bo)]
    )  # [4H]
    with_bias = bool(np.any(bcat))

    # W -> [p, ko, n] bf16, contiguous per partition.
    w_dev = np.ascontiguousarray(
        Wcat.astype(ml_dtypes.bfloat16).reshape(KO, P, NW).transpose(1, 0, 2)
    )

    in_maps = []
    for c in range(N_CORES):
        xc = x[c * B_LOC : (c + 1) * B_LOC]  # [4096, 1024]
        # -> [bg, p, ko, b] so each 512-row slab DMA is contiguous/partition.
        xt_c = np.ascontiguousarray(
            xc.astype(ml_dtypes.bfloat16)
            .reshape(BG, BG_ROWS, KO, P)
            .transpose(0, 3, 2, 1)
        )
        m = {
            "xt": xt_c,
            "w": w_dev,
            "cell": cell[c * B_LOC : (c + 1) * B_LOC],
        }
        if with_bias:
            m["bias"] = np.ascontiguousarray(
                np.broadcast_to(bcat[None, :], (P, NW)).astype(np.float32)
            )
        in_maps.append(m)

    key = with_bias
    if key not in _CACHED:
        _CACHED[key] = _build(with_bias)
    nc = _CACHED[key]

    trace = os.environ.get("KERNEL_TRACE", "0") == "1"
    res = run_bass_kernel_spmd(nc, in_maps, list(range(N_CORES)), trace=trace)
    LAST_RESULTS = res

    h = np.concatenate([res.results[c]["h_out"] for c in range(N_CORES)], axis=0)
    c_ = np.concatenate([res.results[c]["c_out"] for c in range(N_CORES)], axis=0)
    return h, c_


# revision 20
# speedup vs baseline: 1.0078x; 1.0078x over previous
"""CustomLSTM cell (4 gated projections + cell update) on 8 TRN2 NeuronCores.

Data-parallel over the batch dim: each core processes B/8 = 4096 rows.
Per core, z = x @ [Wi|Wf|Wg|Wo] is computed as bf16 matmuls accumulating
f32 into 4 PSUM banks (N = 4*512), the gate nonlinearities run on the
scalar engine straight out of PSUM, and the cell/hidden updates run on
the vector engine.  Host-side prep lays x out transposed ([p, ko, b]
per 512-row batch group) so every DMA is contiguous per partition, and
casts x/W to bf16 (PSUM accumulation stays f32).

Self-contained: shapes/sharding hardcoded for
input [32768, 1024], cell_state [32768, 512], W* [1024, 512].
"""

import os

import numpy as np
import ml_dtypes

import bass_rust
import concourse.bass as bass
import concourse.mybir as mybir
import concourse.tile as tile
from concourse.bass_utils import run_bass_kernel_spmd

N_CORES = 8
B = 32768
D = 1024
H = 512
P = 128
B_LOC = B // N_CORES        # 4096 rows per core
KO = D // P                 # 8 k-subtiles
NW = 4 * H                  # 2048 concatenated gate dim
NG = NW // H                # 4 psum banks of 512
BG_ROWS = 512               # batch rows per x slab
BG = B_LOC // BG_ROWS       # 8 slabs per core
BT_PER_BG = BG_ROWS // P    # 4 batch tiles per slab

BF16 = mybir.dt.bfloat16
F32 = mybir.dt.float32

# Filled by the last kernel() call: BassKernelResults (exec_time_ns etc).
LAST_RESULTS = None
_CACHED = {}


def _split_multi_waits(nc):
    """Legalize for a walrus build that accepts one sync-wait per instruction.

    Tile's wait assignment attaches every needed sem wait to the consuming
    instruction; this backend rejects >1 ("Too many sync wait commands").
    Move all but the last wait onto dedicated NoOps inserted just before the
    instruction on the same engine queue — sequential waits on one engine are
    equivalent to a single multi-wait instruction for monotone sem waits.
    """
    n = 0
    for f in nc.m.functions:
        for blk in f.blocks:
            insts = blk.instructions
            if not any(
                i.sync_info is not None and len(i.sync_info.on_wait) > 1
                for i in insts
            ):
                continue
            out = []
            for inst in insts:
                si = inst.sync_info
                if si is not None and len(si.on_wait) > 1:
                    waits = list(si.on_wait)
                    for w in waits[:-1]:
                        nop = mybir.InstNoOp(name=f"waitsplit_{n}", ins=[], outs=[])
                        n += 1
                        nop.engine = inst.engine
                        nop.sync_info = bass_rust.SyncInfo(on_wait=[w], on_update=[])
                        out.append(nop)
                    inst.sync_info = bass_rust.SyncInfo(
                        on_wait=[waits[-1]], on_update=list(si.on_update)
                    )
                out.append(inst)
            blk.instructions = out


class _FastTailTileContext(tile.TileContext):
    """Drop the second tail all-engine barrier.

    The stock tail is [drain+waits][barrier][sem/queue reset][barrier]; the
    final barrier only isolates the gpsimd-side reset from code that would
    follow it — nothing follows it here, and NRT waits for every engine
    stream (including gpsimd's reset) to halt before completion, so engines
    can end right after the first barrier. Saves ~4-6us of EVSEM ring.
    """

    def _drain_and_barrier(self, tick_clock, wait_clock):
        from concourse.vector_clock import ScopedClock

        drain_inst = self.nc.sync.drain()
        wait_clock.add_sem_waits(
            drain_inst.ins, ScopedClock({None: tick_clock.global_clock})
        )
        self.nc.all_engine_barrier()
        assert self.sems is not None
        popped = self.nc._tile_sem_poison_stack.pop()
        assert popped is self._sem_poison
        self.nc.clear_and_free_semaphores(list(self.sems.allocated().values()))


def _build(with_bias):
    nc = bass.Bass()
    AF = mybir.ActivationFunctionType
    ts = bass.ts

    xt = nc.dram_tensor("xt", [BG, P, KO, BG_ROWS], BF16, kind="ExternalInput")
    w = nc.dram_tensor("w", [P, KO, NW], BF16, kind="ExternalInput")
    cell = nc.dram_tensor("cell", [B_LOC, H], F32, kind="ExternalInput")
    if with_bias:
        bias = nc.dram_tensor("bias", [P, NW], F32, kind="ExternalInput")
    h_out = nc.dram_tensor("h_out", [B_LOC, H], F32, kind="ExternalOutput")
    c_out = nc.dram_tensor("c_out", [B_LOC, H], F32, kind="ExternalOutput")

    with _FastTailTileContext(nc) as tc:
        with (
            tc.tile_pool(name="wpool", bufs=1) as wpool,
            tc.tile_pool(name="xpool", bufs=2) as xpool,
            tc.tile_pool(name="cpool", bufs=4) as cpool,
            tc.tile_pool(name="gpool", bufs=3) as gpool,
            tc.tile_pool(name="ppool", bufs=8, space="PSUM") as ppool,
        ):
            bias_t = None
            if with_bias:
                bias_t = wpool.tile([P, NW], F32, tag="bias_t", name="bias_t")
                nc.sync.dma_start(bias_t[:], bias[:])

            def epilogue(ps, ct, rows, uid, splits=1):
                # gates from psum -> cell/hidden update -> DMA out.
                # splits>1 pipelines the serial ACT->DVE->DMA chain in column
                # chunks — used for the last batch tile to shorten the tail.
                if with_bias:
                    zs = []
                    for nn in range(NG):
                        z = gpool.tile([P, H], F32, tag=f"z{nn}", name=f"z{nn}_{uid}")
                        nc.vector.tensor_add(z[:], ps[nn], bias_t[:, ts(nn, H)])
                        zs.append(z)
                else:
                    zs = ps
                w_ = H // splits
                for q in range(splits):
                    cs = slice(q * w_, (q + 1) * w_)
                    i_t = gpool.tile([P, w_], F32, tag="i_t", name=f"i_{uid}_{q}")
                    nc.scalar.activation(i_t[:], zs[0][:, cs], AF.Sigmoid)
                    f_t = gpool.tile([P, w_], F32, tag="f_t", name=f"f_{uid}_{q}")
                    nc.scalar.activation(f_t[:], zs[1][:, cs], AF.Sigmoid)
                    g_t = gpool.tile([P, w_], F32, tag="g_t", name=f"g_{uid}_{q}")
                    nc.scalar.activation(g_t[:], zs[2][:, cs], AF.Tanh)
                    o_t = gpool.tile([P, w_], F32, tag="o_t", name=f"o_{uid}_{q}")
                    nc.scalar.activation(o_t[:], zs[3][:, cs], AF.Sigmoid)

                    fc = gpool.tile([P, w_], F32, tag="fc", name=f"fc_{uid}_{q}")
                    nc.vector.tensor_mul(fc[:], f_t[:], ct[:, cs])
                    ig = gpool.tile([P, w_], F32, tag="ig", name=f"ig_{uid}_{q}")
                    nc.vector.tensor_mul(ig[:], i_t[:], g_t[:])
                    cn = gpool.tile([P, w_], F32, tag="cn", name=f"cn_{uid}_{q}")
                    nc.vector.tensor_add(cn[:], fc[:], ig[:])
                    tn = gpool.tile([P, w_], F32, tag="tn", name=f"tn_{uid}_{q}")
                    nc.scalar.activation(tn[:], cn[:], AF.Tanh)
                    hn = gpool.tile([P, w_], F32, tag="hn", name=f"hn_{uid}_{q}")
                    nc.vector.tensor_mul(hn[:], o_t[:], tn[:])

                    nc.sync.dma_start(c_out[rows, cs], cn[:])
                    nc.sync.dma_start(h_out[rows, cs], hn[:])

            # PE warmup: ~3.4us of zero matmuls with no DMA dependency so the
            # HAM clock gate opens to 2.4GHz while the first loads are in
            # flight. Results land in a psum slot that bt0 overwrites.
            wz = wpool.tile([P, P], BF16, tag="wz", name="wz")
            nc.gpsimd.memset(wz[:], 0.0)
            warm_ps = ppool.tile([P, P], F32, tag="ps", name="warm_ps")
            for _ in range(16):
                nc.tensor.matmul(warm_ps[:], wz[:], wz[:], start=True, stop=True)

            # Startup DMAs in consumption order: the single HW DMA stream
            # delivers ~356GB/s in trigger order (and each trigger costs
            # ~0.6us on the Sync queue, so fewer/bigger is better past the
            # first chunk). xs0_k rides ahead of its W chunk so the first
            # LDWEIGHTS fires as early as possible.
            # Startup DMAs in consumption order. The HWDGE queues share HBM
            # bandwidth round-robin, so the first W chunk lands ~3us after
            # its trigger; the x chunks ride the (otherwise idle) ACT
            # trigger queue so Sync's serial trigger stream only carries W.
            wks, xs0 = [], []
            for k in range(KO):
                x0k = wpool.tile([P, BG_ROWS], BF16, tag=f"xs0_{k}", name=f"xs0_{k}")
                nc.sync.dma_start(x0k[:], xt[0, :, k, :])
                xs0.append(x0k)
                wk = wpool.tile([P, NW], BF16, tag=f"w{k}", name=f"w{k}")
                nc.sync.dma_start(wk[:], w[:, k, :])
                wks.append(wk)

            def w_slice(k, nn):
                return wks[k][:, ts(nn, H)]

            # Slab 0: k-major over j-pairs so PE consumes each W chunk as it
            # arrives instead of stalling for the whole 4MB of W.
            cts0 = []
            for j in range(BT_PER_BG):
                ct = cpool.tile([P, H], F32, tag="ct", name=f"ct0_{j}")
                nc.sync.dma_start(ct[:], cell[j * P : (j + 1) * P, :])
                cts0.append(ct)
            for jp in (0, 2):
                ps2 = {
                    (j, nn): ppool.tile([P, H], F32, tag="ps", name=f"ps0_{j}_{nn}")
                    for j in (jp, jp + 1)
                    for nn in range(NG)
                }
                for k in range(KO):
                    for j in (jp, jp + 1):
                        lhsT = xs0[k][:, ts(j, P)]
                        for nn in range(NG):
                            nc.tensor.matmul(
                                ps2[(j, nn)],
                                lhsT,
                                w_slice(k, nn),
                                start=(k == 0),
                                stop=(k == KO - 1),
                            )
                for j in (jp, jp + 1):
                    epilogue(
                        [ps2[(j, nn)] for nn in range(NG)],
                        cts0[j],
                        slice(j * P, (j + 1) * P),
                        f"g0_{j}",
                    )

            # Slabs 1..7: j-major, full-rate PE against prefetched slabs.
            for g in range(1, BG):
                xs = xpool.tile([P, KO, BG_ROWS], BF16, tag="xs", name="xs")
                nc.sync.dma_start(xs[:], xt[g])
                for j in range(BT_PER_BG):
                    bt = g * BT_PER_BG + j
                    rows = slice(bt * P, (bt + 1) * P)
                    ct = cpool.tile([P, H], F32, tag="ct", name=f"ct_{bt}")
                    nc.sync.dma_start(ct[:], cell[rows, :])
                    ps = [
                        ppool.tile([P, H], F32, tag="ps", name=f"ps{nn}_{bt}")
                        for nn in range(NG)
                    ]
                    last_bt = g == BG - 1 and j == BT_PER_BG - 1
                    if last_bt:
                        # Bank-by-bank (k-inner) so gates g/i/f are ready
                        # before the final o-bank matmul: the post-MM tail
                        # shrinks to sigmoid(o) -> h -> DMA.
                        for nn in (2, 0, 1, 3):
                            for k in range(KO):
                                nc.tensor.matmul(
                                    ps[nn],
                                    xs[:, k, ts(j, P)],
                                    w_slice(k, nn),
                                    start=(k == 0),
                                    stop=(k == KO - 1),
                                )
                    else:
                        for k in range(KO):
                            lhsT = xs[:, k, ts(j, P)]
                            for nn in range(NG):
                                nc.tensor.matmul(
                                    ps[nn],
                                    lhsT,
                                    w_slice(k, nn),
                                    start=(k == 0),
                                    stop=(k == KO - 1),
                                )
                    epilogue(ps, ct, rows, f"g{g}_{j}")

    _split_multi_waits(nc)
    return nc


def kernel(input, cell_state, Wi, bi, Wf, bf, Wg, bg, Wo, bo):
    global LAST_RESULTS

    x = np.asarray(input, dtype=np.float32)
    cell = np.ascontiguousarray(np.asarray(cell_state, dtype=np.float32))
    Wcat = np.concatenate(
        [np.asarray(m, dtype=np.float32) for m in (Wi, Wf, Wg, Wo)], axis=1
    )  # [D, 4H]
    bcat = np.concatenate(
        [np.asarray(v, dtype=np.float32) for v in (bi, bf, bg, bo)]
    )  # [4H]
    with_bias = bool(np.any(bcat))

    # W -> [p, ko, n] bf16, contiguous per partition.
    w_dev = np.ascontiguousarray(
        Wcat.astype(ml_dtypes.bfloat16).reshape(KO, P, NW).transpose(1, 0, 2)
    )

    in_maps = []
    for c in range(N_CORES):
        xc = x[c * B_LOC : (c + 1) * B_LOC]  # [4096, 1024]
        # -> [bg, p, ko, b] so each 512-row slab DMA is contiguous/partition.
        xt_c = np.ascontiguousarray(
            xc.astype(ml_dtypes.bfloat16)
            .reshape(BG, BG_ROWS, KO, P)
            .transpose(0, 3, 2, 1)
        )
        m = {
            "xt": xt_c,
            "w": w_dev,
            "cell": cell[c * B_LOC : (c + 1) * B_LOC],
        }
        if with_bias:
            m["bias"] = np.ascontiguousarray(
                np.broadcast_to(bcat[None, :], (P, NW)).astype(np.float32)
            )
        in_maps.append(m)

    key = with_bias
    if key not in _CACHED:
        _CACHED[key] = _build(with_bias)
    nc = _CACHED[key]

    trace = os.environ.get("KERNEL_TRACE", "0") == "1"
    res = run_bass_kernel_spmd(nc, in_maps, list(range(N_CORES)), trace=trace)
    LAST_RESULTS = res

    h = np.concatenate([res.results[c]["h_out"] for c in range(N_CORES)], axis=0)
    c_ = np.concatenate([res.results[c]["c_out"] for c in range(N_CORES)], axis=0)
    return h, c_
